# revision 10
# baseline (speedup 1.0000x reference)
import sys
sys.path.insert(0, "/opt/trn_rl_repo")
import numpy as np
import ml_dtypes

import concourse.bass as bass
import concourse.tile as tile
from concourse import bacc, mybir
from concourse.bass_utils import run_bass_kernel_spmd
from concourse import library_config

B, D_IN, D_SAE, K = 4096, 2304, 32768, 64
NC = 8
FS = D_SAE // NC          # 4096 features per core
RS = B // NC              # 512 rows per core in decode
KA = 2304                 # contraction dim
KT = KA // 128            # 18
NSUP = 2                  # feature superblocks per core
NSB = FS // NSUP // 512   # 4 n-tiles (512) per superblock
MT = B // 128             # 32 m tiles
WIN = 8                   # exact-recompute window (ranks 61..68)
KEEP = 64 - WIN // 2      # top-60 kept from fp32r ranking

F32, F32R, BF16, U16, I16 = (mybir.dt.float32, mybir.dt.float32r,
                             mybir.dt.bfloat16, mybir.dt.uint16, mybir.dt.int16)


def _rnd11(a):
    """fp32r grid: round-to-nearest-even to 11 mantissa bits."""
    b = a.view(np.uint32).astype(np.int64)
    low = b & 0xFFF
    base = b & ~0xFFF
    r = base + np.where((low > 2048) | ((low == 2048) & (((b >> 12) & 1) == 1)), 4096, 0)
    return (r & 0xFFFFFFFF).astype(np.uint32).view(np.float32).reshape(a.shape)


def _wrap16(idx):
    """dma_gather index layout: g -> [g%16, g//16], replicated to 128 partitions."""
    n = idx.shape[0]
    lay = np.zeros((16, n // 16), np.int16)
    g = np.arange(n)
    lay[g % 16, g // 16] = idx
    return np.tile(lay, (8, 1))


_cache = {}


def _build_k1():
    nc = bacc.Bacc("TRN2", target_bir_lowering=False, debug=False, num_devices=NC)
    xT_d = nc.dram_tensor("xT", [KA, B], F32R, kind="ExternalInput").ap()
    W_d = nc.dram_tensor("Wsh", [KA, FS], F32R, kind="ExternalInput").ap()
    b_d = nc.dram_tensor("bsh", [1, FS], F32R, kind="ExternalInput").ap()
    ones_d = nc.dram_tensor("ones", [1, 128], F32R, kind="ExternalInput").ap()
    oval = nc.dram_tensor("cand_val", [B, 64], F32, kind="ExternalOutput").ap()
    opos = nc.dram_tensor("cand_pos", [B, 64], U16, kind="ExternalOutput").ap()

    with tile.TileContext(nc) as tc:
        with (
            tc.tile_pool(name="wp", bufs=1) as wp,
            tc.tile_pool(name="xp", bufs=3) as xp,
            tc.tile_pool(name="cp", bufs=3) as cp,
            tc.tile_pool(name="sp", bufs=6) as sp,
            tc.tile_pool(name="ps", bufs=8, space="PSUM") as ps,
        ):
            FSUP = FS // NSUP  # 2048
            ones = wp.tile([1, 128], F32R, tag="ones")
            nc.sync.dma_start(ones[:], ones_d)
            for nsup in range(NSUP):
                w = wp.tile([128, KT * FSUP], F32R, tag="w")
                # load W superblock: [KA, FSUP] -> [p, (kt f)]
                wv = W_d[:, nsup * FSUP:(nsup + 1) * FSUP].rearrange("(kt p) f -> p kt f", p=128)
                nc.sync.dma_start(w.rearrange("p (kt f) -> p kt f", kt=KT)[:], wv)
                bsb = wp.tile([1, FSUP], F32R, tag="bsb")
                nc.sync.dma_start(bsb[:], b_d[:, nsup * FSUP:(nsup + 1) * FSUP])
                for m in range(MT):
                    xt = xp.tile([128, KT * 128], F32R, tag="xt")
                    xv = xT_d[:, m * 128:(m + 1) * 128].rearrange("(kt p) f -> p kt f", p=128)
                    nc.sync.dma_start(xt.rearrange("p (kt f) -> p kt f", kt=KT)[:], xv)
                    cv = cp.tile([128, NSB * 8], F32, tag="cv")
                    cpos = cp.tile([128, NSB * 8], U16, tag="cpos")
                    for n4 in range(NSB):
                        acc = ps.tile([128, 512], F32, tag="acc")
                        for kt in range(KT):
                            nc.tensor.matmul(
                                acc[:],
                                xt[:, kt * 128:(kt + 1) * 128],
                                w[:, kt * FSUP + n4 * 512: kt * FSUP + n4 * 512 + 512],
                                start=(kt == 0),
                                stop=False,
                            )
                        nc.tensor.matmul(
                            acc[:], ones[:],
                            bsb[:, n4 * 512:(n4 + 1) * 512],
                            start=False, stop=True,
                        )
                        nc.vector.max(cv[:, n4 * 8:(n4 + 1) * 8], acc[:])
                        nc.vector.max_index(cpos[:, n4 * 8:(n4 + 1) * 8], cv[:, n4 * 8:(n4 + 1) * 8], acc[:])
                    rs = slice(m * 128, (m + 1) * 128)
                    cs = slice(nsup * NSB * 8, (nsup + 1) * NSB * 8)
                    nc.sync.dma_start(oval[rs, cs], cv[:])
                    nc.sync.dma_start(opos[rs, cs], cpos[:])
    nc.compile()
    return nc


def _build_k2():
    nc = bacc.Bacc("TRN2", target_bir_lowering=False, debug=False, num_devices=NC)
    Wd_d = nc.dram_tensor("Wdec", [D_SAE, D_IN], BF16, kind="ExternalInput").ap()
    idx_d = nc.dram_tensor("idxs", [32, 128, 64], I16, kind="ExternalInput").ap()
    val_d = nc.dram_tensor("vals", [64, 128, 4], F32, kind="ExternalInput").ap()
    bd_d = nc.dram_tensor("bdec", [128, D_IN], F32, kind="ExternalInput").ap()
    out_d = nc.dram_tensor("xhat", [RS, D_IN], F32, kind="ExternalOutput").ap()

    with tile.TileContext(nc) as tc:
        with (
            tc.tile_pool(name="sb", bufs=1) as sb,
            tc.tile_pool(name="gp", bufs=3) as gp,
        ):
            nc.gpsimd.load_library(library_config.mlp)
            idxs = sb.tile([128, 32 * 64], I16, tag="idxs")
            nc.sync.dma_start(idxs.rearrange("p (k c) -> p k c", k=32)[:], idx_d.rearrange("k p c -> p k c"))
            vals = sb.tile([128, 64 * 4], F32, tag="vals")
            nc.sync.dma_start(vals.rearrange("p (k c) -> p k c", k=64)[:], val_d.rearrange("k p c -> p k c"))
            bd = sb.tile([128, D_IN], F32, tag="bd")
            nc.sync.dma_start(bd[:], bd_d)
            accs = []
            for bb in range(4):
                a = sb.tile([128, D_IN], F32, tag=f"acc{bb}")
                nc.vector.tensor_copy(a[:], bd[:])
                accs.append(a)
            gsem = nc.alloc_semaphore("gsem")
            for it in range(16):
                ga = gp.tile([128, 8 * D_IN], BF16, tag="g")
                gb = gp.tile([128, 8 * D_IN], BF16, tag="g")
                with tc.tile_critical():
                    # two gathers per critical: descriptor-gen of the second
                    # overlaps the first's DMA flight
                    nc.gpsimd.dma_gather(
                        ga.rearrange("p (j e) -> p j e", j=8)[:], Wd_d,
                        idxs[:, (2 * it) * 64:(2 * it + 1) * 64],
                        num_idxs=1024, num_idxs_reg=1024, elem_size=D_IN,
                    ).then_inc(gsem, 16)
                    nc.gpsimd.dma_gather(
                        gb.rearrange("p (j e) -> p j e", j=8)[:], Wd_d,
                        idxs[:, (2 * it + 1) * 64:(2 * it + 2) * 64],
                        num_idxs=1024, num_idxs_reg=1024, elem_size=D_IN,
                    ).then_inc(gsem, 16)
                    nc.gpsimd.wait_ge(gsem, 32 * (it + 1))
                for half, g in ((0, ga), (1, gb)):
                    for kk in range(2):
                        k = (2 * it + half) * 2 + kk
                        for bb in range(4):
                            nc.vector.scalar_tensor_tensor(
                                accs[bb][:], g[:, (kk * 4 + bb) * D_IN:(kk * 4 + bb + 1) * D_IN],
                                vals[:, k * 4 + bb: k * 4 + bb + 1], accs[bb][:],
                                op0=mybir.AluOpType.mult, op1=mybir.AluOpType.add,
                            )
            for bb in range(4):
                nc.sync.dma_start(out_d[bb * 128:(bb + 1) * 128, :], accs[bb][:])
    nc.compile()
    return nc


def kernel(x, W_enc, W_dec, b_enc, b_dec):
    x = np.asarray(x, dtype=np.float32)
    W_enc = np.asarray(W_enc, dtype=np.float32)
    W_dec = np.asarray(W_dec, dtype=np.float32)
    b_enc = np.asarray(b_enc, dtype=np.float32)
    b_dec = np.asarray(b_dec, dtype=np.float32)

    # ---- host prep: fold bias row, round to fp32r grid, transpose ----
    xt = x - b_dec                                   # [B, D_IN]
    xTa = _rnd11(np.ascontiguousarray(xt.T))
    Wa = _rnd11(W_enc)
    ba = _rnd11(b_enc)[None, :]

    if "k1" not in _cache:
        _cache["k1"] = _build_k1()
    k1 = _cache["k1"]
    in_maps = [{"xT": xTa, "Wsh": np.ascontiguousarray(Wa[:, c * FS:(c + 1) * FS]),
                "bsh": np.ascontiguousarray(ba[:, c * FS:(c + 1) * FS]),
                "ones": np.ones((1, 128), np.float32)}
               for c in range(NC)]
    import time as _time
    _t0 = _time.time()
    res1 = run_bass_kernel_spmd(k1, in_maps, list(range(NC)))
    _cache["t1_wall"] = _time.time() - _t0

    # ---- host merge: global candidate sort + exact boundary fixup ----
    cand_val = np.concatenate([res1.results[c]["cand_val"] for c in range(NC)], axis=1)  # [B, 512]
    cand_pos = np.concatenate([res1.results[c]["cand_pos"] for c in range(NC)], axis=1).astype(np.int64)
    col = np.arange(512)[None, :]
    core = col // 64
    chunk = (col % 64) // 8
    cand_gidx = core * FS + chunk * 512 + cand_pos     # [B, 512]

    order = np.argsort(-cand_val, axis=1, kind="stable")[:, :KEEP + WIN]
    s_val = np.take_along_axis(cand_val, order, axis=1)
    s_idx = np.take_along_axis(cand_gidx, order, axis=1)

    # exact recompute of window ranks [KEEP, KEEP+WIN)
    w_idx = s_idx[:, KEEP:]                            # [B, WIN]
    WT = np.ascontiguousarray(W_enc.T)                 # [D_SAE, D_IN]
    w_exact = np.einsum("rd,rkd->rk", xt, WT[w_idx], optimize=True) + b_enc[w_idx]
    o = np.argsort(-w_exact, axis=1, kind="stable")[:, :64 - KEEP]
    fix_idx = np.take_along_axis(w_idx, o, axis=1)
    fix_val = np.take_along_axis(w_exact, o, axis=1)

    sel_idx = np.concatenate([s_idx[:, :KEEP], fix_idx], axis=1)      # [B, 64]
    sel_val = np.maximum(np.concatenate([s_val[:, :KEEP], fix_val], axis=1), 0.0).astype(np.float32)

    # ---- build decode layouts ----
    if "k2" not in _cache:
        _cache["k2"] = _build_k2()
        _cache["Wdec_b"] = None
    k2 = _cache["k2"]
    Wdec_b = W_dec.astype(ml_dtypes.bfloat16)
    bdec_rep = np.tile(b_dec[None, :], (128, 1)).astype(np.float32)

    in_maps2 = []
    for c in range(NC):
        rs = slice(c * RS, (c + 1) * RS)
        si = sel_idx[rs].astype(np.int16)              # [RS, 64]
        sv = sel_val[rs]                               # [RS, 64]
        idxs = np.stack([_wrap16(si[:, 2 * i:2 * i + 2].T.ravel()) for i in range(32)])  # [32, 128, 64]
        vals = np.ascontiguousarray(
            sv.reshape(4, 128, 64).transpose(2, 1, 0)).astype(np.float32)  # [64, 128, 4]
        in_maps2.append({"Wdec": Wdec_b, "idxs": idxs, "vals": vals, "bdec": bdec_rep})
    _t0 = _time.time()
    res2 = run_bass_kernel_spmd(k2, in_maps2, list(range(NC)))
    _cache["t2_wall"] = _time.time() - _t0

    xhat = np.concatenate([res2.results[c]["xhat"] for c in range(NC)], axis=0)
    return xhat
